# revision 5
# baseline (speedup 1.0000x reference)
"""Trainium2 kernel for the DepthTracker correlation pyramid.

Math: for each level l, frame t, track n, the reference bilinearly samples a
7x7 grid of points around coords[t,n] from fmaps_l (128 channels) and
correlates each sample with the 49 track features -> out (L,B,T,N,7,7,7,7).

Decomposition used here (verified to 4e-7 rel err vs the jax reference):
  out[l,t,n,h,w,pq] = sum_{u,v} Sx[l,t,n,h,v] * Sy[l,t,n,w,u] * G[l,n,pq,t,u,v]
  G[l,n,pq,t,uv]    = sum_c trackT[c,(l,n,pq)] * patch[l,n,c,(t,uv)]
where patch is the 8x8 integer-pixel support window whose origin is
clip(floor(coord)-3, 0, dim-8) and Sx/Sy are 7x8 sparse bilinear blend
matrices (border clamping folded in on the host).

The device does the heavy part: G = track^T @ patches, a (49x128)@(128x1024)
matmul per (level, track), 10 GFLOP total, track-stationary so the PE streams
512-wide fp32r columns at full rate. Tracks are sharded 32-per-core across
the 8 NeuronCores (fully data parallel). The tiny blend (0.6 GFLOP) and the
final transpose run on the host.
"""

import numpy as np

R = 3
K7 = 7
LEV = 4
B, T, C, N = 1, 16, 128, 256
H, W = 96, 128
NCORES = 8
NS = N // NCORES          # 32 tracks per core
UV = 64                   # 8x8 support window
TUV = T * UV              # 1024
PQ = K7 * K7              # 49
HALF = TUV // 2           # 512, one PSUM bank of fp32

COMPUTE_DT = 'f32r'       # 'f32r' | 'f32' | 'f16' | 'bf16'
TRACE = False             # set True to capture an NTFF profile (test.py only)
LAST_RESULT = {}          # phase timings + profile info for test.py

_BASS_CACHE = {}


def _np_compute_dtype():
    if COMPUTE_DT in ('f32r', 'f32'):
        return np.float32
    if COMPUTE_DT == 'f16':
        return np.float16
    import ml_dtypes
    return np.dtype(ml_dtypes.bfloat16)


def _build_bass():
    key = COMPUTE_DT
    if key in _BASS_CACHE:
        return _BASS_CACHE[key]
    import concourse.bacc as bacc
    import concourse.mybir as mybir
    from concourse import tile

    cdt = {
        'f32r': mybir.dt.float32r,
        'f32': mybir.dt.float32,
        'f16': mybir.dt.float16,
        'bf16': mybir.dt.bfloat16,
    }[COMPUTE_DT]
    f32 = mybir.dt.float32

    nc = bacc.Bacc("TRN2", target_bir_lowering=False, debug=False)
    patches = nc.dram_tensor("patches", (LEV, NS, C, TUV), cdt,
                             kind="ExternalInput")
    trackT = nc.dram_tensor("trackT", (C, LEV * NS * PQ), cdt,
                            kind="ExternalInput")
    gout = nc.dram_tensor("gout", (LEV, NS, PQ, TUV), f32,
                          kind="ExternalOutput")

    with tile.TileContext(nc) as tc:
        with (
            tc.tile_pool(name="track", bufs=1) as track_pool,
            tc.tile_pool(name="patch", bufs=6) as patch_pool,
            tc.tile_pool(name="out", bufs=4) as out_pool,
            tc.tile_pool(name="psum", bufs=4, space="PSUM") as psum_pool,
        ):
            tr = track_pool.tile([C, LEV * NS * PQ], cdt)
            nc.sync.dma_start(tr[:], trackT[:])
            for l in range(LEV):
                for n in range(NS):
                    pt = patch_pool.tile([C, TUV], cdt, tag="pt")
                    nc.sync.dma_start(pt[:], patches[l, n])
                    ot = out_pool.tile([PQ, TUV], f32, tag="ot")
                    w = tr[:, (l * NS + n) * PQ:(l * NS + n + 1) * PQ]
                    for h in range(2):
                        ps = psum_pool.tile([PQ, HALF], f32, tag="ps")
                        nc.tensor.matmul(
                            ps[:], w, pt[:, h * HALF:(h + 1) * HALF],
                            start=True, stop=True)
                        # split the PSUM->SBUF copies across DVE and ACT
                        if h == 0:
                            nc.vector.tensor_copy(ot[:, 0:HALF], ps[:])
                        else:
                            nc.scalar.copy(ot[:, HALF:TUV], ps[:])
                    nc.sync.dma_start(gout[l, n], ot[:])
    nc.compile()
    _BASS_CACHE[key] = nc
    return nc


def _blend_mats(xy, dim):
    """xy: (T,N) fp32 coords at this level's scale. Returns (origin (T,N)
    int32, S (T,N,7,8) fp32) with reference clamping semantics folded in."""
    d = np.arange(-R, R + 1, dtype=np.float32)
    q = xy[..., None] + d
    qc = np.clip(q, 0.0, dim - 1.0)
    x0 = np.floor(qc)
    w = (qc - x0).astype(np.float32)
    x0i = x0.astype(np.int32)
    x1i = np.minimum(x0i + 1, dim - 1)
    org = np.clip(np.floor(xy).astype(np.int32) - R, 0, dim - 8)
    v0 = x0i - org[..., None]
    v1 = x1i - org[..., None]
    eye = np.eye(8, dtype=np.float32)
    S = eye[v0] * (1.0 - w)[..., None] + eye[v1] * w[..., None]
    return org, S


def kernel(fmaps0, fmaps1, fmaps2, fmaps3, track0, track1, track2, track3,
           coords):
    import time as _time
    _t0 = _time.time()
    fmaps = [fmaps0, fmaps1, fmaps2, fmaps3]
    tracks = [track0, track1, track2, track3]
    cdt_np = _np_compute_dtype()
    coords2 = np.asarray(coords, np.float32)[0]        # (T,N,2)

    # ---- host: blend matrices + patch gather --------------------------------
    patches_all = np.empty((LEV, N, C, T, 8, 8), cdt_np)
    Sx_all = np.empty((LEV, T, N, K7, 8), np.float32)
    Sy_all = np.empty((LEV, T, N, K7, 8), np.float32)
    for l in range(LEV):
        Hl, Wl = H >> l, W >> l
        sc = np.float32(2.0 ** l)
        x = (coords2[..., 0] / sc).astype(np.float32)
        y = (coords2[..., 1] / sc).astype(np.float32)
        cx, Sx_all[l] = _blend_mats(x, Wl)
        cy, Sy_all[l] = _blend_mats(y, Hl)
        fm = np.asarray(fmaps[l], np.float32)[0]       # (T,C,Hl,Wl)
        iy = cy[..., None] + np.arange(8)              # (T,N,8)
        ix = cx[..., None] + np.arange(8)
        t_idx = np.arange(T)[:, None, None, None]
        # fancy indexing -> (T,N,8,8,C)
        p = fm[t_idx, :, iy[:, :, :, None], ix[:, :, None, :]]
        patches_all[l] = p.transpose(1, 4, 0, 2, 3)    # (N,C,T,8,8)

    trackT_all = np.empty((C, LEV, N, PQ), cdt_np)
    for l in range(LEV):
        # track_l: (1,49,N,C) -> (C, N, PQ)
        trackT_all[:, l] = np.asarray(tracks[l], np.float32)[0].transpose(2, 1, 0)

    # ---- device: G = track^T @ patches, 32 tracks per core ------------------
    nc = _build_bass()
    from concourse import bass_utils
    in_maps = []
    for k in range(NCORES):
        sl = slice(k * NS, (k + 1) * NS)
        in_maps.append({
            "patches": np.ascontiguousarray(
                patches_all[:, sl].reshape(LEV, NS, C, TUV)),
            "trackT": np.ascontiguousarray(
                trackT_all[:, :, sl].reshape(C, LEV * NS * PQ)),
        })
    _t1 = _time.time()
    res = bass_utils.run_bass_kernel_spmd(
        nc, in_maps, core_ids=list(range(NCORES)), trace=TRACE)
    _t2 = _time.time()
    LAST_RESULT.update(
        host_pre_s=_t1 - _t0, spmd_s=_t2 - _t1,
        exec_time_ns=res.exec_time_ns, profile_json=res.profile_json)
    # (LEV, NCORES, NS, PQ, TUV) -> (LEV, N, PQ, T, 8, 8)
    G = np.stack([r["gout"] for r in res.results], axis=1)
    G = G.reshape(LEV, N, PQ, T, 8, 8)

    # ---- host: separable bilinear blend + final layout ----------------------
    # step 1: contract u (rows):  Y[l,t,n,w,pq,v] = sum_u Sy[...,w,u]*G[...]
    Gt = np.ascontiguousarray(G.transpose(0, 3, 1, 4, 2, 5))   # (L,T,N,8,PQ,8)
    Y = np.matmul(Sy_all, Gt.reshape(LEV, T, N, 8, PQ * 8))    # (L,T,N,7,PQ*8)
    Y = Y.reshape(LEV, T, N, K7, PQ, 8)
    # step 2: contract v (cols): out[l,t,n,h,w,pq] = sum_v Sx[...,h,v]*Y[...]
    Yt = np.ascontiguousarray(Y.transpose(0, 1, 2, 5, 3, 4))   # (L,T,N,8,7,PQ)
    V = np.matmul(Sx_all, Yt.reshape(LEV, T, N, 8, K7 * PQ))   # (L,T,N,7,7*PQ)
    V = V.reshape(LEV, T, N, K7, K7, K7, K7)
    out = V.reshape(LEV, B, T, N, K7, K7, K7, K7).astype(np.float32)
    LAST_RESULT['host_post_s'] = _time.time() - _t2
    return out


# revision 10
# speedup vs baseline: 1.2366x; 1.2366x over previous
"""Trainium2 kernel for the DepthTracker correlation pyramid.

Math: for each level l, frame t, track n, the reference bilinearly samples a
7x7 grid of points around coords[t,n] from fmaps_l (128 channels) and
correlates each sample with the 49 track features -> out (L,B,T,N,7,7,7,7).

Decomposition used here (verified to 4e-7 rel err vs the jax reference):
  out[l,t,n,h,w,pq] = sum_{u,v} Sx[l,t,n,h,v] * Sy[l,t,n,w,u] * G[l,n,pq,t,u,v]
  G[l,n,pq,t,uv]    = sum_c trackT[c,(l,n,pq)] * patch[l,n,c,(t,uv)]
where patch is the 8x8 integer-pixel support window whose origin is
clip(floor(coord)-3, 0, dim-8) and Sx/Sy are 7x8 sparse bilinear blend
matrices (border clamping folded in on the host).

The device does the heavy part: G = track^T @ patches, a (49x128)@(128x1024)
matmul per (level, track), 10 GFLOP total, track-stationary so the PE streams
512-wide fp32r columns at full rate. Tracks are sharded 32-per-core across
the 8 NeuronCores (fully data parallel). The tiny blend (0.6 GFLOP) and the
final transpose run on the host.
"""

import numpy as np

R = 3
K7 = 7
LEV = 4
B, T, C, N = 1, 16, 128, 256
H, W = 96, 128
NCORES = 8
NS = N // NCORES          # 32 tracks per core
UV = 64                   # 8x8 support window
TUV = T * UV              # 1024
PQ = K7 * K7              # 49
HALF = TUV // 2           # 512, one PSUM bank of fp32

COMPUTE_DT = 'f32r'       # 'f32r' | 'f32' | 'f16' | 'bf16'
OUT_DT = 'f32'            # dtype of the device G output: 'f32' | 'f16'
TRACE = False             # set True to capture an NTFF profile (test.py only)
LAST_RESULT = {}          # phase timings + profile info for test.py

_BASS_CACHE = {}


def _np_compute_dtype():
    if COMPUTE_DT in ('f32r', 'f32'):
        return np.float32
    if COMPUTE_DT == 'f16':
        return np.float16
    import ml_dtypes
    return np.dtype(ml_dtypes.bfloat16)


def _build_bass():
    key = (COMPUTE_DT, OUT_DT)
    if key in _BASS_CACHE:
        return _BASS_CACHE[key]
    import concourse.bacc as bacc
    import concourse.mybir as mybir
    from concourse import tile

    cdt = {
        'f32r': mybir.dt.float32r,
        'f32': mybir.dt.float32,
        'f16': mybir.dt.float16,
        'bf16': mybir.dt.bfloat16,
    }[COMPUTE_DT]
    f32 = mybir.dt.float32
    odt = f32 if OUT_DT == 'f32' else mybir.dt.float16

    nc = bacc.Bacc("TRN2", target_bir_lowering=False, debug=False)
    patches = nc.dram_tensor("patches", (LEV, NS, C, TUV), cdt,
                             kind="ExternalInput")
    trackT = nc.dram_tensor("trackT", (C, LEV * NS * PQ), cdt,
                            kind="ExternalInput")
    gout = nc.dram_tensor("gout", (LEV, NS, PQ, TUV), odt,
                          kind="ExternalOutput")

    with tile.TileContext(nc) as tc:
        with (
            tc.tile_pool(name="track", bufs=1) as track_pool,
            tc.tile_pool(name="patch", bufs=6) as patch_pool,
            tc.tile_pool(name="out", bufs=4) as out_pool,
            tc.tile_pool(name="psum", bufs=4, space="PSUM") as psum_pool,
        ):
            tr = track_pool.tile([C, LEV * NS * PQ], cdt)
            nc.sync.dma_start(tr[:], trackT[:])
            for l in range(LEV):
                for n in range(NS):
                    pt = patch_pool.tile([C, TUV], cdt, tag="pt")
                    nc.sync.dma_start(pt[:], patches[l, n])
                    ot = out_pool.tile([PQ, TUV], odt, tag="ot")
                    w = tr[:, (l * NS + n) * PQ:(l * NS + n + 1) * PQ]
                    for h in range(2):
                        ps = psum_pool.tile([PQ, HALF], f32, tag="ps")
                        nc.tensor.matmul(
                            ps[:], w, pt[:, h * HALF:(h + 1) * HALF],
                            start=True, stop=True)
                        # split the PSUM->SBUF copies across DVE and ACT
                        if h == 0:
                            nc.vector.tensor_copy(ot[:, 0:HALF], ps[:])
                        else:
                            nc.scalar.copy(ot[:, HALF:TUV], ps[:])
                    nc.sync.dma_start(gout[l, n], ot[:])
    nc.compile()
    _BASS_CACHE[key] = nc
    return nc


def _blend_mats(xy, dim):
    """xy: (T,N) fp32 coords at this level's scale. Returns (origin (T,N)
    int32, S (T,N,7,8) fp32) with reference clamping semantics folded in."""
    d = np.arange(-R, R + 1, dtype=np.float32)
    q = xy[..., None] + d
    qc = np.clip(q, 0.0, dim - 1.0)
    x0 = np.floor(qc)
    w = (qc - x0).astype(np.float32)
    x0i = x0.astype(np.int32)
    x1i = np.minimum(x0i + 1, dim - 1)
    org = np.clip(np.floor(xy).astype(np.int32) - R, 0, dim - 8)
    v0 = x0i - org[..., None]
    v1 = x1i - org[..., None]
    eye = np.eye(8, dtype=np.float32)
    S = eye[v0] * (1.0 - w)[..., None] + eye[v1] * w[..., None]
    return org, S


def kernel(fmaps0, fmaps1, fmaps2, fmaps3, track0, track1, track2, track3,
           coords):
    import time as _time
    _t0 = _time.time()
    fmaps = [fmaps0, fmaps1, fmaps2, fmaps3]
    tracks = [track0, track1, track2, track3]
    cdt_np = _np_compute_dtype()
    coords2 = np.asarray(coords, np.float32)[0]        # (T,N,2)

    # ---- host: blend matrices + patch gather --------------------------------
    patches_all = np.empty((LEV, N, C, T, 8, 8), cdt_np)
    Sx_all = np.empty((LEV, T, N, K7, 8), np.float32)
    Sy_all = np.empty((LEV, T, N, K7, 8), np.float32)
    for l in range(LEV):
        Hl, Wl = H >> l, W >> l
        sc = np.float32(2.0 ** l)
        x = (coords2[..., 0] / sc).astype(np.float32)
        y = (coords2[..., 1] / sc).astype(np.float32)
        cx, Sx_all[l] = _blend_mats(x, Wl)
        cy, Sy_all[l] = _blend_mats(y, Hl)
        fm = np.asarray(fmaps[l], np.float32)[0]       # (T,C,Hl,Wl)
        iy = cy[..., None] + np.arange(8)              # (T,N,8)
        ix = cx[..., None] + np.arange(8)
        t_idx = np.arange(T)[:, None, None, None]
        # fancy indexing -> (T,N,8,8,C)
        p = fm[t_idx, :, iy[:, :, :, None], ix[:, :, None, :]]
        patches_all[l] = p.transpose(1, 4, 0, 2, 3)    # (N,C,T,8,8)

    trackT_all = np.empty((C, LEV, N, PQ), cdt_np)
    for l in range(LEV):
        # track_l: (1,49,N,C) -> (C, N, PQ)
        trackT_all[:, l] = np.asarray(tracks[l], np.float32)[0].transpose(2, 1, 0)

    # ---- device: G = track^T @ patches, 32 tracks per core ------------------
    nc = _build_bass()
    from concourse import bass_utils
    in_maps = []
    for k in range(NCORES):
        sl = slice(k * NS, (k + 1) * NS)
        in_maps.append({
            "patches": np.ascontiguousarray(
                patches_all[:, sl].reshape(LEV, NS, C, TUV)),
            "trackT": np.ascontiguousarray(
                trackT_all[:, :, sl].reshape(C, LEV * NS * PQ)),
        })
    _t1 = _time.time()
    res = bass_utils.run_bass_kernel_spmd(
        nc, in_maps, core_ids=list(range(NCORES)), trace=TRACE)
    _t2 = _time.time()
    LAST_RESULT.update(
        host_pre_s=_t1 - _t0, spmd_s=_t2 - _t1,
        exec_time_ns=res.exec_time_ns, profile_json=res.profile_json)
    # (LEV, NCORES, NS, PQ, TUV) -> (LEV, N, PQ, T, 8, 8)
    G = np.stack([r["gout"] for r in res.results], axis=1)
    G = G.reshape(LEV, N, PQ, T, 8, 8).astype(np.float32)

    # ---- host: separable bilinear blend + final layout ----------------------
    # step 1: contract u (rows):  Y[l,t,n,w,pq,v] = sum_u Sy[...,w,u]*G[...]
    Gt = np.ascontiguousarray(G.transpose(0, 3, 1, 4, 2, 5))   # (L,T,N,8,PQ,8)
    Y = np.matmul(Sy_all, Gt.reshape(LEV, T, N, 8, PQ * 8))    # (L,T,N,7,PQ*8)
    Y = Y.reshape(LEV, T, N, K7, PQ, 8)
    # step 2: contract v (cols): out[l,t,n,h,w,pq] = sum_v Sx[...,h,v]*Y[...]
    Yt = np.ascontiguousarray(Y.transpose(0, 1, 2, 5, 3, 4))   # (L,T,N,8,7,PQ)
    V = np.matmul(Sx_all, Yt.reshape(LEV, T, N, 8, K7 * PQ))   # (L,T,N,7,7*PQ)
    V = V.reshape(LEV, T, N, K7, K7, K7, K7)
    out = V.reshape(LEV, B, T, N, K7, K7, K7, K7).astype(np.float32)
    LAST_RESULT['host_post_s'] = _time.time() - _t2
    return out


# revision 13
# speedup vs baseline: 1.6619x; 1.3439x over previous
"""Trainium2 kernel for the DepthTracker correlation pyramid.

Math: for each level l, frame t, track n, the reference bilinearly samples a
7x7 grid of points around coords[t,n] from fmaps_l (128 channels) and
correlates each sample with the 49 track features -> out (L,B,T,N,7,7,7,7).

Decomposition used here (verified to 4e-7 rel err vs the jax reference):
  out[l,t,n,h,w,pq] = sum_{u,v} Sx[l,t,n,h,v] * Sy[l,t,n,w,u] * G[l,n,pq,t,u,v]
  G[l,n,pq,t,uv]    = sum_c trackT[c,(l,n,pq)] * patch[l,n,c,(t,uv)]
where patch is the 8x8 integer-pixel support window whose origin is
clip(floor(coord)-3, 0, dim-8) and Sx/Sy are 7x8 sparse bilinear blend
matrices (border clamping folded in on the host).

The device does the heavy part: G = track^T @ patches, a (49x128)@(128x1024)
matmul per (level, track), 10 GFLOP total, track-stationary so the PE streams
512-wide fp32r columns at full rate. Tracks are sharded 32-per-core across
the 8 NeuronCores (fully data parallel). The tiny blend (0.6 GFLOP) and the
final transpose run on the host.
"""

import numpy as np

R = 3
K7 = 7
LEV = 4
B, T, C, N = 1, 16, 128, 256
H, W = 96, 128
NCORES = 8
NS = N // NCORES          # 32 tracks per core
UV = 64                   # 8x8 support window
TUV = T * UV              # 1024
PQ = K7 * K7              # 49
HALF = TUV // 2           # 512, one PSUM bank of fp32

COMPUTE_DT = 'f32r'       # 'f32r' | 'f32' | 'f16' | 'bf16'
OUT_DT = 'f32'            # dtype of the device G output: 'f32' | 'f16'
TRACE = False             # set True to capture an NTFF profile (test.py only)
LAST_RESULT = {}          # phase timings + profile info for test.py

_BASS_CACHE = {}


def _np_compute_dtype():
    if COMPUTE_DT in ('f32r', 'f32'):
        return np.float32
    if COMPUTE_DT == 'f16':
        return np.float16
    import ml_dtypes
    return np.dtype(ml_dtypes.bfloat16)


def _build_bass():
    key = (COMPUTE_DT, OUT_DT)
    if key in _BASS_CACHE:
        return _BASS_CACHE[key]
    import concourse.bacc as bacc
    import concourse.mybir as mybir
    from concourse import tile

    cdt = {
        'f32r': mybir.dt.float32r,
        'f32': mybir.dt.float32,
        'f16': mybir.dt.float16,
        'bf16': mybir.dt.bfloat16,
    }[COMPUTE_DT]
    f32 = mybir.dt.float32
    odt = f32 if OUT_DT == 'f32' else mybir.dt.float16

    nc = bacc.Bacc("TRN2", target_bir_lowering=False, debug=False)
    patches = nc.dram_tensor("patches", (LEV, NS, C, TUV), cdt,
                             kind="ExternalInput")
    trackT = nc.dram_tensor("trackT", (C, LEV * NS * PQ), cdt,
                            kind="ExternalInput")
    gout = nc.dram_tensor("gout", (LEV, NS, PQ, TUV), odt,
                          kind="ExternalOutput")

    NB = 4  # tracks per DMA batch
    with tile.TileContext(nc) as tc:
        with (
            tc.tile_pool(name="track", bufs=1) as track_pool,
            tc.tile_pool(name="patch", bufs=3) as patch_pool,
            tc.tile_pool(name="out", bufs=3) as out_pool,
            tc.tile_pool(name="psum", bufs=4, space="PSUM") as psum_pool,
        ):
            tr = track_pool.tile([C, LEV * NS * PQ], cdt)
            nc.sync.dma_start(tr[:], trackT[:])
            for l in range(LEV):
                for nb in range(NS // NB):
                    pt = patch_pool.tile([C, NB * TUV], cdt, tag="pt")
                    nc.sync.dma_start(
                        pt[:].rearrange("c (g v) -> c g v", g=NB),
                        patches[l, nb * NB:(nb + 1) * NB].rearrange(
                            "g c v -> c g v"))
                    ot = out_pool.tile([PQ, NB * TUV], odt, tag="ot")
                    for g in range(NB):
                        n = nb * NB + g
                        w = tr[:, (l * NS + n) * PQ:(l * NS + n + 1) * PQ]
                        for h in range(2):
                            ps = psum_pool.tile([PQ, HALF], f32, tag="ps")
                            nc.tensor.matmul(
                                ps[:], w,
                                pt[:, g * TUV + h * HALF:
                                      g * TUV + (h + 1) * HALF],
                                start=True, stop=True)
                            # split the PSUM->SBUF copies across DVE and ACT
                            dst = ot[:, g * TUV + h * HALF:
                                        g * TUV + (h + 1) * HALF]
                            if h == 0:
                                nc.vector.tensor_copy(dst, ps[:])
                            else:
                                nc.scalar.copy(dst, ps[:])
                    # store on the ACT HWDGE ring (loads use the SP ring)
                    nc.scalar.dma_start(
                        gout[l, nb * NB:(nb + 1) * NB].rearrange(
                            "g p v -> p g v"),
                        ot[:].rearrange("p (g v) -> p g v", g=NB))
    nc.compile()
    _BASS_CACHE[key] = nc
    return nc


def _blend_mats(xy, dim):
    """xy: (T,N) fp32 coords at this level's scale. Returns (origin (T,N)
    int32, S (T,N,7,8) fp32) with reference clamping semantics folded in."""
    d = np.arange(-R, R + 1, dtype=np.float32)
    q = xy[..., None] + d
    qc = np.clip(q, 0.0, dim - 1.0)
    x0 = np.floor(qc)
    w = (qc - x0).astype(np.float32)
    x0i = x0.astype(np.int32)
    x1i = np.minimum(x0i + 1, dim - 1)
    org = np.clip(np.floor(xy).astype(np.int32) - R, 0, dim - 8)
    v0 = x0i - org[..., None]
    v1 = x1i - org[..., None]
    eye = np.eye(8, dtype=np.float32)
    S = eye[v0] * (1.0 - w)[..., None] + eye[v1] * w[..., None]
    return org, S


def kernel(fmaps0, fmaps1, fmaps2, fmaps3, track0, track1, track2, track3,
           coords):
    import time as _time
    _t0 = _time.time()
    fmaps = [fmaps0, fmaps1, fmaps2, fmaps3]
    tracks = [track0, track1, track2, track3]
    cdt_np = _np_compute_dtype()
    coords2 = np.asarray(coords, np.float32)[0]        # (T,N,2)

    # ---- host: blend matrices + patch gather --------------------------------
    patches_all = np.empty((LEV, N, C, T, 8, 8), cdt_np)
    Sx_all = np.empty((LEV, T, N, K7, 8), np.float32)
    Sy_all = np.empty((LEV, T, N, K7, 8), np.float32)
    for l in range(LEV):
        Hl, Wl = H >> l, W >> l
        sc = np.float32(2.0 ** l)
        x = (coords2[..., 0] / sc).astype(np.float32)
        y = (coords2[..., 1] / sc).astype(np.float32)
        cx, Sx_all[l] = _blend_mats(x, Wl)
        cy, Sy_all[l] = _blend_mats(y, Hl)
        fm = np.asarray(fmaps[l], np.float32)[0]       # (T,C,Hl,Wl)
        iy = cy[..., None] + np.arange(8)              # (T,N,8)
        ix = cx[..., None] + np.arange(8)
        t_idx = np.arange(T)[:, None, None, None]
        # fancy indexing -> (T,N,8,8,C)
        p = fm[t_idx, :, iy[:, :, :, None], ix[:, :, None, :]]
        patches_all[l] = p.transpose(1, 4, 0, 2, 3)    # (N,C,T,8,8)

    trackT_all = np.empty((C, LEV, N, PQ), cdt_np)
    for l in range(LEV):
        # track_l: (1,49,N,C) -> (C, N, PQ)
        trackT_all[:, l] = np.asarray(tracks[l], np.float32)[0].transpose(2, 1, 0)

    # ---- device: G = track^T @ patches, 32 tracks per core ------------------
    nc = _build_bass()
    from concourse import bass_utils
    in_maps = []
    for k in range(NCORES):
        sl = slice(k * NS, (k + 1) * NS)
        in_maps.append({
            "patches": np.ascontiguousarray(
                patches_all[:, sl].reshape(LEV, NS, C, TUV)),
            "trackT": np.ascontiguousarray(
                trackT_all[:, :, sl].reshape(C, LEV * NS * PQ)),
        })
    _t1 = _time.time()
    res = bass_utils.run_bass_kernel_spmd(
        nc, in_maps, core_ids=list(range(NCORES)), trace=TRACE)
    _t2 = _time.time()
    LAST_RESULT.update(
        host_pre_s=_t1 - _t0, spmd_s=_t2 - _t1,
        exec_time_ns=res.exec_time_ns, profile_json=res.profile_json)
    # (LEV, NCORES, NS, PQ, TUV) -> (LEV, N, PQ, T, 8, 8)
    G = np.stack([r["gout"] for r in res.results], axis=1)
    G = G.reshape(LEV, N, PQ, T, 8, 8).astype(np.float32)

    # ---- host: separable bilinear blend + final layout ----------------------
    # step 1: contract u (rows):  Y[l,t,n,w,pq,v] = sum_u Sy[...,w,u]*G[...]
    Gt = np.ascontiguousarray(G.transpose(0, 3, 1, 4, 2, 5))   # (L,T,N,8,PQ,8)
    Y = np.matmul(Sy_all, Gt.reshape(LEV, T, N, 8, PQ * 8))    # (L,T,N,7,PQ*8)
    Y = Y.reshape(LEV, T, N, K7, PQ, 8)
    # step 2: contract v (cols): out[l,t,n,h,w,pq] = sum_v Sx[...,h,v]*Y[...]
    Yt = np.ascontiguousarray(Y.transpose(0, 1, 2, 5, 3, 4))   # (L,T,N,8,7,PQ)
    V = np.matmul(Sx_all, Yt.reshape(LEV, T, N, 8, K7 * PQ))   # (L,T,N,7,7*PQ)
    V = V.reshape(LEV, T, N, K7, K7, K7, K7)
    out = V.reshape(LEV, B, T, N, K7, K7, K7, K7).astype(np.float32)
    LAST_RESULT['host_post_s'] = _time.time() - _t2
    return out


# revision 16
# speedup vs baseline: 1.7823x; 1.0725x over previous
"""Trainium2 kernel for the DepthTracker correlation pyramid.

Math: for each level l, frame t, track n, the reference bilinearly samples a
7x7 grid of points around coords[t,n] from fmaps_l (128 channels) and
correlates each sample with the 49 track features -> out (L,B,T,N,7,7,7,7).

Decomposition used here (verified to 4e-7 rel err vs the jax reference):
  out[l,t,n,h,w,pq] = sum_{u,v} Sx[l,t,n,h,v] * Sy[l,t,n,w,u] * G[l,n,pq,t,u,v]
  G[l,n,pq,t,uv]    = sum_c trackT[c,(l,n,pq)] * patch[l,n,c,(t,uv)]
where patch is the 8x8 integer-pixel support window whose origin is
clip(floor(coord)-3, 0, dim-8) and Sx/Sy are 7x8 sparse bilinear blend
matrices (border clamping folded in on the host).

The device does the heavy part: G = track^T @ patches, a (49x128)@(128x1024)
matmul per (level, track), 10 GFLOP total, track-stationary so the PE streams
512-wide fp32r columns at full rate. Tracks are sharded 32-per-core across
the 8 NeuronCores (fully data parallel). The tiny blend (0.6 GFLOP) and the
final transpose run on the host.
"""

import numpy as np

R = 3
K7 = 7
LEV = 4
B, T, C, N = 1, 16, 128, 256
H, W = 96, 128
NCORES = 8
NS = N // NCORES          # 32 tracks per core
UV = 64                   # 8x8 support window
TUV = T * UV              # 1024
PQ = K7 * K7              # 49
HALF = TUV // 2           # 512, one PSUM bank of fp32

COMPUTE_DT = 'f32r'       # 'f32r' | 'f32' | 'f16' | 'bf16'
OUT_DT = 'f32'            # dtype of the device G output: 'f32' | 'f16'
TRACE = False             # set True to capture an NTFF profile (test.py only)
LAST_RESULT = {}          # phase timings + profile info for test.py

_BASS_CACHE = {}


def _np_compute_dtype():
    if COMPUTE_DT in ('f32r', 'f32'):
        return np.float32
    if COMPUTE_DT == 'f16':
        return np.float16
    import ml_dtypes
    return np.dtype(ml_dtypes.bfloat16)


def _build_bass():
    key = (COMPUTE_DT, OUT_DT)
    if key in _BASS_CACHE:
        return _BASS_CACHE[key]
    import concourse.bacc as bacc
    import concourse.mybir as mybir
    from concourse import tile

    cdt = {
        'f32r': mybir.dt.float32r,
        'f32': mybir.dt.float32,
        'f16': mybir.dt.float16,
        'bf16': mybir.dt.bfloat16,
    }[COMPUTE_DT]
    f32 = mybir.dt.float32
    odt = f32 if OUT_DT == 'f32' else mybir.dt.float16

    nc = bacc.Bacc("TRN2", target_bir_lowering=False, debug=False)
    patches = nc.dram_tensor("patches", (LEV, NS, C, TUV), cdt,
                             kind="ExternalInput")
    trackT = nc.dram_tensor("trackT", (C, LEV * NS * PQ), cdt,
                            kind="ExternalInput")
    gout = nc.dram_tensor("gout", (LEV, NS, PQ, TUV), odt,
                          kind="ExternalOutput")

    NB = 8  # tracks per DMA batch
    with tile.TileContext(nc) as tc:
        with (
            tc.tile_pool(name="track", bufs=1) as track_pool,
            tc.tile_pool(name="patch", bufs=3) as patch_pool,
            tc.tile_pool(name="out", bufs=3) as out_pool,
            tc.tile_pool(name="psum", bufs=4, space="PSUM") as psum_pool,
        ):
            tr = track_pool.tile([C, LEV * NS * PQ], cdt)
            nc.sync.dma_start(tr[:], trackT[:])
            for l in range(LEV):
                for nb in range(NS // NB):
                    pt = patch_pool.tile([C, NB * TUV], cdt, tag="pt")
                    nc.sync.dma_start(
                        pt[:].rearrange("c (g v) -> c g v", g=NB),
                        patches[l, nb * NB:(nb + 1) * NB].rearrange(
                            "g c v -> c g v"))
                    ot = out_pool.tile([PQ, NB * TUV], odt, tag="ot")
                    for g in range(NB):
                        n = nb * NB + g
                        w = tr[:, (l * NS + n) * PQ:(l * NS + n + 1) * PQ]
                        ps = psum_pool.tile([PQ, TUV], f32, tag="ps")
                        for h in range(2):
                            nc.tensor.matmul(
                                ps[:, h * HALF:(h + 1) * HALF], w,
                                pt[:, g * TUV + h * HALF:
                                      g * TUV + (h + 1) * HALF],
                                start=True, stop=True)
                        # one fused PSUM->SBUF copy per track,
                        # alternating between DVE and ACT
                        dst = ot[:, g * TUV:(g + 1) * TUV]
                        if g % 2 == 0:
                            nc.vector.tensor_copy(dst, ps[:])
                        else:
                            nc.scalar.copy(dst, ps[:])
                    # store on the ACT HWDGE ring (loads use the SP ring)
                    nc.scalar.dma_start(
                        gout[l, nb * NB:(nb + 1) * NB].rearrange(
                            "g p v -> p g v"),
                        ot[:].rearrange("p (g v) -> p g v", g=NB))
    nc.compile()
    _BASS_CACHE[key] = nc
    return nc


def _blend_mats(xy, dim):
    """xy: (T,N) fp32 coords at this level's scale. Returns (origin (T,N)
    int32, S (T,N,7,8) fp32) with reference clamping semantics folded in."""
    d = np.arange(-R, R + 1, dtype=np.float32)
    q = xy[..., None] + d
    qc = np.clip(q, 0.0, dim - 1.0)
    x0 = np.floor(qc)
    w = (qc - x0).astype(np.float32)
    x0i = x0.astype(np.int32)
    x1i = np.minimum(x0i + 1, dim - 1)
    org = np.clip(np.floor(xy).astype(np.int32) - R, 0, dim - 8)
    v0 = x0i - org[..., None]
    v1 = x1i - org[..., None]
    eye = np.eye(8, dtype=np.float32)
    S = eye[v0] * (1.0 - w)[..., None] + eye[v1] * w[..., None]
    return org, S


def kernel(fmaps0, fmaps1, fmaps2, fmaps3, track0, track1, track2, track3,
           coords):
    import time as _time
    _t0 = _time.time()
    fmaps = [fmaps0, fmaps1, fmaps2, fmaps3]
    tracks = [track0, track1, track2, track3]
    cdt_np = _np_compute_dtype()
    coords2 = np.asarray(coords, np.float32)[0]        # (T,N,2)

    # ---- host: blend matrices + patch gather --------------------------------
    patches_all = np.empty((LEV, N, C, T, 8, 8), cdt_np)
    Sx_all = np.empty((LEV, T, N, K7, 8), np.float32)
    Sy_all = np.empty((LEV, T, N, K7, 8), np.float32)
    for l in range(LEV):
        Hl, Wl = H >> l, W >> l
        sc = np.float32(2.0 ** l)
        x = (coords2[..., 0] / sc).astype(np.float32)
        y = (coords2[..., 1] / sc).astype(np.float32)
        cx, Sx_all[l] = _blend_mats(x, Wl)
        cy, Sy_all[l] = _blend_mats(y, Hl)
        fm = np.asarray(fmaps[l], np.float32)[0]       # (T,C,Hl,Wl)
        iy = cy[..., None] + np.arange(8)              # (T,N,8)
        ix = cx[..., None] + np.arange(8)
        t_idx = np.arange(T)[:, None, None, None]
        # fancy indexing -> (T,N,8,8,C)
        p = fm[t_idx, :, iy[:, :, :, None], ix[:, :, None, :]]
        patches_all[l] = p.transpose(1, 4, 0, 2, 3)    # (N,C,T,8,8)

    trackT_all = np.empty((C, LEV, N, PQ), cdt_np)
    for l in range(LEV):
        # track_l: (1,49,N,C) -> (C, N, PQ)
        trackT_all[:, l] = np.asarray(tracks[l], np.float32)[0].transpose(2, 1, 0)

    # ---- device: G = track^T @ patches, 32 tracks per core ------------------
    nc = _build_bass()
    from concourse import bass_utils
    in_maps = []
    for k in range(NCORES):
        sl = slice(k * NS, (k + 1) * NS)
        in_maps.append({
            "patches": np.ascontiguousarray(
                patches_all[:, sl].reshape(LEV, NS, C, TUV)),
            "trackT": np.ascontiguousarray(
                trackT_all[:, :, sl].reshape(C, LEV * NS * PQ)),
        })
    _t1 = _time.time()
    res = bass_utils.run_bass_kernel_spmd(
        nc, in_maps, core_ids=list(range(NCORES)), trace=TRACE)
    _t2 = _time.time()
    LAST_RESULT.update(
        host_pre_s=_t1 - _t0, spmd_s=_t2 - _t1,
        exec_time_ns=res.exec_time_ns, profile_json=res.profile_json)
    # (LEV, NCORES, NS, PQ, TUV) -> (LEV, N, PQ, T, 8, 8)
    G = np.stack([r["gout"] for r in res.results], axis=1)
    G = G.reshape(LEV, N, PQ, T, 8, 8).astype(np.float32)

    # ---- host: separable bilinear blend + final layout ----------------------
    # step 1: contract u (rows):  Y[l,t,n,w,pq,v] = sum_u Sy[...,w,u]*G[...]
    Gt = np.ascontiguousarray(G.transpose(0, 3, 1, 4, 2, 5))   # (L,T,N,8,PQ,8)
    Y = np.matmul(Sy_all, Gt.reshape(LEV, T, N, 8, PQ * 8))    # (L,T,N,7,PQ*8)
    Y = Y.reshape(LEV, T, N, K7, PQ, 8)
    # step 2: contract v (cols): out[l,t,n,h,w,pq] = sum_v Sx[...,h,v]*Y[...]
    Yt = np.ascontiguousarray(Y.transpose(0, 1, 2, 5, 3, 4))   # (L,T,N,8,7,PQ)
    V = np.matmul(Sx_all, Yt.reshape(LEV, T, N, 8, K7 * PQ))   # (L,T,N,7,7*PQ)
    V = V.reshape(LEV, T, N, K7, K7, K7, K7)
    out = V.reshape(LEV, B, T, N, K7, K7, K7, K7).astype(np.float32)
    LAST_RESULT['host_post_s'] = _time.time() - _t2
    return out
